# revision 2
# baseline (speedup 1.0000x reference)
"""Grouped-GEMM MoE experts (E=64, H=2048, F=1408, 16 tokens/expert, SwiGLU),
expert-parallel across 8 Trainium2 NeuronCores - int8 weight streaming.

The bf16 baseline is DMA-bound (138.4 MB of weights/core at ~334-380 GB/s).
This version halves HBM traffic: weights are quantized to int8 on host
(global per-tensor symmetric scales; rel-err ~0.74% vs 2e-2 tolerance) and
dequantized on-chip to bf16 (int8->bf16 copy is exact for |v|<=127).

HW-measured rates that drive the design:
  - sync HWDGE ring: ~334 GB/s traced, 2.88 MB pair transfers (the sweet
    spot; per-unit 1.44 MB transfers measured 6% slower). Adding a SWDGE
    channel does NOT add bandwidth and cast-during-DMA costs 2x stream
    bytes -> both rejected after measurement.
  - DVE tensor_copy (CAST): 2 elem/cyc/lane = 245 G/s -> cols [0:6976)
  - ACT copy (ACTIVATE):    1 elem/cyc/lane = 153 G/s -> cols [6976:11264)
  - GPSIMD: ~4 cyc/elem and its SBUF-port contention slows DVE 3x -> unused
  - PE (FWL weight-stationary gate/up, inter-stationary down): ~178us busy
All four streams (DMA ~210us, ACT ~202, DVE ~192, PE ~178) are within ~15%.
The first pair of expert 0 and the last pair of expert 7 stream in quarter
pieces (0.72 MB) so the dequant+PE pipeline ramps and drains quickly.

Scale bookkeeping (all folded outside the kernel):
  host:  x_tilde = x * s1 (bf16), weights stored as round(W/s) int8
  device: g = Q1^T x_tilde  (true gate values -> silu OK)
          u_raw = Q3^T x_tilde = (s1/s3) u
          y_raw = (silu(g) * u_raw)^T Q2
  host:  y = y_raw * (s2*s3/s1)
"""

import sys

if "/opt/trn_rl_repo" not in sys.path:
    sys.path.insert(0, "/opt/trn_rl_repo")

import numpy as np
import ml_dtypes

E, H, F = 64, 2048, 1408
TOK = 16                  # tokens per expert (uniform routing)
NCORES = 8
E_LOC = E // NCORES       # 8 experts per core
T_LOC = E_LOC * TOK       # 128 tokens per core
P = 128
HC = H // P               # 16 contraction chunks for gate/up
FC = F // P               # 11 contraction chunks for down
UCOLS = 2 * F * 4         # 11264 cols per weight unit
PAIR = 2 * UCOLS          # 22528 cols per int8 DMA pair (2.88 MB)
NFREE = 512               # matmul max free dim = one PSUM bank
BF16 = ml_dtypes.bfloat16
CW = 2 * F                # 2816 cols per gate/up h-chunk

VW = 6976                 # DVE dequant cols per unit (ACT gets the rest)

_cache = {}


def _build_nc():
    import concourse.mybir as mybir
    from concourse import bacc
    from concourse.tile import TileContext

    f32 = mybir.dt.float32
    bf16 = mybir.dt.bfloat16
    i8 = mybir.dt.int8
    AF = mybir.ActivationFunctionType

    nc = bacc.Bacc()
    xt_d = nc.declare_dram_parameter("xt", [P, HC * T_LOC], bf16, isOutput=False)
    w_d = nc.declare_dram_parameter("w", [E_LOC, 3, P, PAIR], i8, isOutput=False)
    y_d = nc.declare_dram_parameter("y", [T_LOC, H], f32, isOutput=True)

    with TileContext(nc) as tc:
        with (
            tc.tile_pool(name="xs", bufs=1) as xs,
            tc.tile_pool(name="w8", bufs=3) as w8,
            tc.tile_pool(name="wt", bufs=5) as wt,
            tc.tile_pool(name="acts", bufs=2) as acts,
            tc.tile_pool(name="ps_gu", bufs=2, space="PSUM") as ps_gu,
            tc.tile_pool(name="ps_dn", bufs=1, space="PSUM") as ps_dn,
        ):
            # xt on the scalar HWDGE ring so the weight stream (sync ring)
            # starts immediately; xt is only needed a few us in.
            xt = xs.tile([P, HC * T_LOC], bf16)
            nc.scalar.dma_start(out=xt[:], in_=xt_d[:, :])

            def convert(dst, src, off, lo, hi):
                """Dequant dst[:, lo:hi] from src[:, off+lo : off+hi]:
                DVE below VW, ACT above (measured 2 vs 1 elem/cyc/lane)."""
                if lo < VW:
                    m = min(hi, VW)
                    nc.vector.tensor_copy(out=dst[:, lo:m], in_=src[:, off + lo : off + m])
                if hi > VW:
                    m = max(lo, VW)
                    nc.scalar.copy(out=dst[:, m:hi], in_=src[:, off + m : off + hi])

            for e in range(E_LOC):
                last = e == E_LOC - 1
                # int8 weight stream: 3 pair transfers of [128, 22528] per
                # expert (gu01 | gu23 | dn45), each dequantized into 2 bf16
                # units. The head pair of expert 0 and the tail pair of the
                # last expert stream in quarter pieces so the pipeline ramps
                # in ~2us and the final matmuls chase the last bytes.
                units = []
                for j in range(3):
                    p8 = w8.tile([P, PAIR], i8, tag="p8")
                    fine = False
                    if fine:
                        q = PAIR // 4  # 5632
                        for k in range(4):
                            nc.sync.dma_start(
                                out=p8[:, k * q : (k + 1) * q],
                                in_=w_d[e, j, :, k * q : (k + 1) * q],
                            )
                            if k % 2 == 0:
                                t = wt.tile([P, UCOLS], bf16, tag="w")
                            # strips aligned to each 5632-col piece:
                            # DVE [0:3584]+[5632:9216], ACT [3584:5632]+[9216:]
                            if k % 2 == 0:
                                nc.vector.tensor_copy(
                                    out=t[:, :3584], in_=p8[:, k * q : k * q + 3584]
                                )
                                nc.scalar.copy(
                                    out=t[:, 3584:q], in_=p8[:, k * q + 3584 : (k + 1) * q]
                                )
                            else:
                                nc.vector.tensor_copy(
                                    out=t[:, q:9216], in_=p8[:, k * q : k * q + 3584]
                                )
                                nc.scalar.copy(
                                    out=t[:, 9216:], in_=p8[:, k * q + 3584 : (k + 1) * q]
                                )
                                units.append(t)
                    else:
                        nc.sync.dma_start(out=p8[:], in_=w_d[e, j, :, :])
                        for half in range(2):
                            t = wt.tile([P, UCOLS], bf16, tag="w")
                            convert(t, p8, half * UCOLS, 0, UCOLS)
                            units.append(t)
                gu_map = [(units[c // 4], (c % 4) * CW) for c in range(HC)]
                dn_units = ((units[4], 0, UCOLS), (units[5], UCOLS, PAIR))

                # gate/up: all FC output chunks share one PSUM bank per
                # tensor; only the first matmul into the bank clears it
                # (start=True), later chunks overwrite via has_written.
                gt = ps_gu.tile([P, FC * TOK], f32, tag="gt")
                ut = ps_gu.tile([P, FC * TOK], f32, tag="ut")
                rhs_e = e * TOK
                for c in range(HC):
                    wu, base = gu_map[c]
                    rhs = xt[:, c * T_LOC + rhs_e : c * T_LOC + rhs_e + TOK]
                    first = c == 0
                    final = c == HC - 1
                    w1o = base
                    w3o = base + F
                    for fc in range(FC):
                        nc.tensor.matmul(
                            gt[:, fc * TOK : (fc + 1) * TOK],
                            wu[:, w1o + fc * P : w1o + (fc + 1) * P],
                            rhs,
                            start=(first and fc == 0),
                            stop=(final and fc == FC - 1),
                            skip_group_check=True,
                        )
                    for fc in range(FC):
                        nc.tensor.matmul(
                            ut[:, fc * TOK : (fc + 1) * TOK],
                            wu[:, w3o + fc * P : w3o + (fc + 1) * P],
                            rhs,
                            start=(first and fc == 0),
                            stop=(final and fc == FC - 1),
                            skip_group_check=True,
                        )

                gs = acts.tile([P, FC * TOK], f32, tag="gs")
                it = acts.tile([P, FC * TOK], bf16, tag="it")
                nc.scalar.activation(gs[:], gt[:], AF.Silu)
                nc.vector.tensor_mul(it[:], gs[:], ut[:])

                def dn_src(col):
                    for t, lo, hi in dn_units:
                        if lo <= col < hi:
                            return t, col - lo
                    raise AssertionError

                dn = ps_dn.tile([P, H], f32, tag="dn")
                for fc in range(FC):
                    for nt in range(H // NFREE):
                        col = fc * H + nt * NFREE
                        wu, off = dn_src(col)
                        nc.tensor.matmul(
                            dn[:TOK, nt * NFREE : (nt + 1) * NFREE],
                            it[:, fc * TOK : (fc + 1) * TOK],
                            wu[:, off : off + NFREE],
                            start=(fc == 0),
                            stop=(fc == FC - 1),
                        )

                # copy out per 512-col slice (PSUM bank granularity) so the
                # copies overlap the last f-chunk's matmuls, alternating
                # DVE/ACT to split the load; y DMAs go on the scalar HWDGE
                # ring so they never stall the sync weight ring.
                ob = acts.tile([TOK, H], f32, tag="ob")
                rows = slice(e * TOK, (e + 1) * TOK)
                for nt in range(H // NFREE):
                    cols = slice(nt * NFREE, (nt + 1) * NFREE)
                    if nt % 2 == 0:
                        nc.vector.tensor_copy(out=ob[:, cols], in_=dn[:TOK, cols])
                    else:
                        nc.scalar.copy(out=ob[:, cols], in_=dn[:TOK, cols])
                    if last:
                        nc.scalar.dma_start(out=y_d[rows, cols], in_=ob[:, cols])
                if not last:
                    nc.scalar.dma_start(out=y_d[rows, : H // 2], in_=ob[:, : H // 2])
                    nc.scalar.dma_start(out=y_d[rows, H // 2 :], in_=ob[:, H // 2 :])

    if not nc.is_finalized():
        nc.finalize()
    return nc


def _get_nc():
    if "nc" not in _cache:
        _cache["nc"] = _build_nc()
    return _cache["nc"]


def _pack_core(xs, q1, q3, q2):
    """Pack one core's slice into the kernel's DMA-ready layout.

    xs: pre-scaled x (float32, cast to bf16); q1/q3/q2: int8 weights.
    """
    xt = np.ascontiguousarray(
        xs.reshape(T_LOC, HC, P).transpose(2, 1, 0).reshape(P, HC * T_LOC)
    ).astype(BF16)
    # gate/up units: [e, u, p, (cs, {w1,w3}, f)]
    w1r = q1.reshape(E_LOC, HC, P, F)
    w3r = q3.reshape(E_LOC, HC, P, F)
    gu = np.stack([w1r, w3r], axis=3)               # [e, c, p, s, f]
    gu = gu.reshape(E_LOC, 4, 4, P, 2, F)           # [e, u, cs, p, s, f]
    gu = gu.transpose(0, 1, 3, 2, 4, 5).reshape(E_LOC, 4, P, UCOLS)
    # down units: [e, p, fc*H + h] split into 2 units of UCOLS
    dn = q2.reshape(E_LOC, FC, P, H).transpose(0, 2, 1, 3).reshape(E_LOC, P, 2, UCOLS)
    dn = dn.transpose(0, 2, 1, 3)                   # [e, 2, p, UCOLS]
    w = np.concatenate([gu, dn], axis=1)            # [e, 6, p, UCOLS] int8
    # merge unit pairs so each is one contiguous [128, 22528] int8 DMA:
    # (gu01 | gu23 | dn45)
    w = np.ascontiguousarray(
        w.reshape(E_LOC, 3, 2, P, UCOLS)
        .transpose(0, 1, 3, 2, 4)
        .reshape(E_LOC, 3, P, PAIR)
    )
    return xt, w


def _quant(w):
    s = float(np.abs(w).max()) / 127.0
    q = np.rint(w * (1.0 / s)).astype(np.int8)
    return q, s


def _make_in_maps(inputs):
    x = np.asarray(inputs["permuted_local_hidden_states"], dtype=np.float32)
    w1 = np.asarray(inputs["gate_proj"], dtype=np.float32)
    w3 = np.asarray(inputs["up_proj"], dtype=np.float32)
    w2 = np.asarray(inputs["down_proj"], dtype=np.float32)
    q1, s1 = _quant(w1)
    q3, s3 = _quant(w3)
    q2, s2 = _quant(w2)
    xs = x * np.float32(s1)
    in_maps = []
    for m in range(NCORES):
        xt, w = _pack_core(
            xs[m * T_LOC : (m + 1) * T_LOC],
            q1[m * E_LOC : (m + 1) * E_LOC],
            q3[m * E_LOC : (m + 1) * E_LOC],
            q2[m * E_LOC : (m + 1) * E_LOC],
        )
        in_maps.append({"xt": xt, "w": w})
    return in_maps, np.float32(s2 * s3 / s1)


def run(inputs, trace=False, **kwargs):
    """Run the SPMD kernel; returns (y_full, BassKernelResults)."""
    from concourse.bass_utils import run_bass_kernel_spmd

    nc = _get_nc()
    in_maps, post = _make_in_maps(inputs)
    res = run_bass_kernel_spmd(
        nc, in_maps, list(range(NCORES)), trace=trace, **kwargs
    )
    y = np.concatenate([res.results[m]["y"] for m in range(NCORES)], axis=0)
    return (y * post).astype(np.float32, copy=False), res


def kernel(**inputs):
    y, _ = run(inputs, trace=False)
    return y


# revision 3
# speedup vs baseline: 1.0293x; 1.0293x over previous
"""Grouped-GEMM MoE experts (E=64, H=2048, F=1408, 16 tokens/expert, SwiGLU),
expert-parallel across 8 Trainium2 NeuronCores - int8 weight streaming.

The bf16 baseline is DMA-bound (138.4 MB of weights/core at ~334-380 GB/s).
This version halves HBM traffic: weights are quantized to int8 on host
(global per-tensor symmetric scales; rel-err ~0.74% vs 2e-2 tolerance) and
dequantized on-chip to bf16 (int8->bf16 copy is exact for |v|<=127).

HW-measured rates that drive the design:
  - sync HWDGE ring: ~334 GB/s traced, 2.88 MB pair transfers (the sweet
    spot; per-unit 1.44 MB transfers measured 6% slower). Adding a SWDGE
    channel does NOT add bandwidth and cast-during-DMA costs 2x stream
    bytes -> both rejected after measurement.
  - DVE tensor_copy (CAST): 2 elem/cyc/lane = 245 G/s -> cols [0:6976)
  - ACT copy (ACTIVATE):    1 elem/cyc/lane = 153 G/s -> cols [6976:11264)
  - GPSIMD: ~4 cyc/elem and its SBUF-port contention slows DVE 3x -> unused
  - PE (FWL weight-stationary gate/up, inter-stationary down): ~178us busy
All four streams (DMA ~210us, ACT ~202, DVE ~192, PE ~178) are within ~15%,
and the wall time (~252us traced) sits ~40us above the DMA span (head ramp +
last-expert drain + epilogue). Finer DMA pieces, SWDGE assist, pipelined
out-copies, and hoisting the last expert's gate/up were all measured SLOWER
(257-303us) - this per-expert pair-streamed structure is the local optimum.

Scale bookkeeping (all folded outside the kernel):
  host:  x_tilde = x * s1 (bf16), weights stored as round(W/s) int8
  device: g = Q1^T x_tilde  (true gate values -> silu OK)
          u_raw = Q3^T x_tilde = (s1/s3) u
          y_raw = (silu(g) * u_raw)^T Q2
  host:  y = y_raw * (s2*s3/s1)
"""

import sys

if "/opt/trn_rl_repo" not in sys.path:
    sys.path.insert(0, "/opt/trn_rl_repo")

import numpy as np
import ml_dtypes

E, H, F = 64, 2048, 1408
TOK = 16                  # tokens per expert (uniform routing)
NCORES = 8
E_LOC = E // NCORES       # 8 experts per core
T_LOC = E_LOC * TOK       # 128 tokens per core
P = 128
HC = H // P               # 16 contraction chunks for gate/up
FC = F // P               # 11 contraction chunks for down
UCOLS = 2 * F * 4         # 11264 cols per weight unit
PAIR = 2 * UCOLS          # 22528 cols per int8 DMA pair (2.88 MB)
NFREE = 512               # matmul max free dim = one PSUM bank
BF16 = ml_dtypes.bfloat16
CW = 2 * F                # 2816 cols per gate/up h-chunk

VW = 6976                 # DVE dequant cols per unit (ACT gets the rest)

_cache = {}


def _build_nc():
    import concourse.mybir as mybir
    from concourse import bacc
    from concourse.tile import TileContext

    f32 = mybir.dt.float32
    bf16 = mybir.dt.bfloat16
    i8 = mybir.dt.int8
    AF = mybir.ActivationFunctionType

    nc = bacc.Bacc()
    xt_d = nc.declare_dram_parameter("xt", [P, HC * T_LOC], bf16, isOutput=False)
    w_d = nc.declare_dram_parameter("w", [E_LOC, 3, P, PAIR], i8, isOutput=False)
    y_d = nc.declare_dram_parameter("y", [T_LOC, H], f32, isOutput=True)

    with TileContext(nc) as tc:
        with (
            tc.tile_pool(name="xs", bufs=1) as xs,
            tc.tile_pool(name="w8", bufs=3) as w8,
            tc.tile_pool(name="wt", bufs=5) as wt,
            tc.tile_pool(name="acts", bufs=2) as acts,
            tc.tile_pool(name="ps_gu", bufs=2, space="PSUM") as ps_gu,
            tc.tile_pool(name="ps_dn", bufs=1, space="PSUM") as ps_dn,
        ):
            # xt on the scalar HWDGE ring so the weight stream (sync ring)
            # starts immediately; xt is only needed a few us in.
            xt = xs.tile([P, HC * T_LOC], bf16)
            nc.scalar.dma_start(out=xt[:], in_=xt_d[:, :])

            def convert(dst, src, off, lo, hi):
                """Dequant dst[:, lo:hi] from src[:, off+lo : off+hi]:
                DVE below VW, ACT above (measured 2 vs 1 elem/cyc/lane)."""
                if lo < VW:
                    m = min(hi, VW)
                    nc.vector.tensor_copy(out=dst[:, lo:m], in_=src[:, off + lo : off + m])
                if hi > VW:
                    m = max(lo, VW)
                    nc.scalar.copy(out=dst[:, m:hi], in_=src[:, off + m : off + hi])

            for e in range(E_LOC):
                last = e == E_LOC - 1
                # int8 weight stream: 3 pair transfers of [128, 22528] per
                # expert (gu01 | gu23 | dn45), each dequantized into 2 bf16
                # units. The head pair of expert 0 and the tail pair of the
                # last expert stream in quarter pieces so the pipeline ramps
                # in ~2us and the final matmuls chase the last bytes.
                units = []
                for j in range(3):
                    p8 = w8.tile([P, PAIR], i8, tag="p8")
                    fine = False
                    if fine:
                        q = PAIR // 4  # 5632
                        for k in range(4):
                            nc.sync.dma_start(
                                out=p8[:, k * q : (k + 1) * q],
                                in_=w_d[e, j, :, k * q : (k + 1) * q],
                            )
                            if k % 2 == 0:
                                t = wt.tile([P, UCOLS], bf16, tag="w")
                            # strips aligned to each 5632-col piece:
                            # DVE [0:3584]+[5632:9216], ACT [3584:5632]+[9216:]
                            if k % 2 == 0:
                                nc.vector.tensor_copy(
                                    out=t[:, :3584], in_=p8[:, k * q : k * q + 3584]
                                )
                                nc.scalar.copy(
                                    out=t[:, 3584:q], in_=p8[:, k * q + 3584 : (k + 1) * q]
                                )
                            else:
                                nc.vector.tensor_copy(
                                    out=t[:, q:9216], in_=p8[:, k * q : k * q + 3584]
                                )
                                nc.scalar.copy(
                                    out=t[:, 9216:], in_=p8[:, k * q + 3584 : (k + 1) * q]
                                )
                                units.append(t)
                    else:
                        nc.sync.dma_start(out=p8[:], in_=w_d[e, j, :, :])
                        for half in range(2):
                            t = wt.tile([P, UCOLS], bf16, tag="w")
                            convert(t, p8, half * UCOLS, 0, UCOLS)
                            units.append(t)
                gu_map = [(units[c // 4], (c % 4) * CW) for c in range(HC)]
                dn_units = ((units[4], 0, UCOLS), (units[5], UCOLS, PAIR))

                # gate/up: all FC output chunks share one PSUM bank per
                # tensor; only the first matmul into the bank clears it
                # (start=True), later chunks overwrite via has_written.
                gt = ps_gu.tile([P, FC * TOK], f32, tag="gt")
                ut = ps_gu.tile([P, FC * TOK], f32, tag="ut")
                rhs_e = e * TOK
                for c in range(HC):
                    wu, base = gu_map[c]
                    rhs = xt[:, c * T_LOC + rhs_e : c * T_LOC + rhs_e + TOK]
                    first = c == 0
                    final = c == HC - 1
                    w1o = base
                    w3o = base + F
                    for fc in range(FC):
                        nc.tensor.matmul(
                            gt[:, fc * TOK : (fc + 1) * TOK],
                            wu[:, w1o + fc * P : w1o + (fc + 1) * P],
                            rhs,
                            start=(first and fc == 0),
                            stop=(final and fc == FC - 1),
                            skip_group_check=True,
                        )
                    for fc in range(FC):
                        nc.tensor.matmul(
                            ut[:, fc * TOK : (fc + 1) * TOK],
                            wu[:, w3o + fc * P : w3o + (fc + 1) * P],
                            rhs,
                            start=(first and fc == 0),
                            stop=(final and fc == FC - 1),
                            skip_group_check=True,
                        )

                gs = acts.tile([P, FC * TOK], f32, tag="gs")
                it = acts.tile([P, FC * TOK], bf16, tag="it")
                nc.scalar.activation(gs[:], gt[:], AF.Silu)
                nc.vector.tensor_mul(it[:], gs[:], ut[:])

                def dn_src(col):
                    for t, lo, hi in dn_units:
                        if lo <= col < hi:
                            return t, col - lo
                    raise AssertionError

                dn = ps_dn.tile([P, H], f32, tag="dn")
                for fc in range(FC):
                    for nt in range(H // NFREE):
                        col = fc * H + nt * NFREE
                        wu, off = dn_src(col)
                        nc.tensor.matmul(
                            dn[:TOK, nt * NFREE : (nt + 1) * NFREE],
                            it[:, fc * TOK : (fc + 1) * TOK],
                            wu[:, off : off + NFREE],
                            start=(fc == 0),
                            stop=(fc == FC - 1),
                        )

                # copy out per 512-col slice (PSUM bank granularity) so the
                # copies overlap the last f-chunk's matmuls, alternating
                # DVE/ACT to split the load; y DMAs go on the scalar HWDGE
                # ring so they never stall the sync weight ring.
                ob = acts.tile([TOK, H], f32, tag="ob")
                rows = slice(e * TOK, (e + 1) * TOK)
                for nt in range(H // NFREE):
                    cols = slice(nt * NFREE, (nt + 1) * NFREE)
                    if nt % 2 == 0:
                        nc.vector.tensor_copy(out=ob[:, cols], in_=dn[:TOK, cols])
                    else:
                        nc.scalar.copy(out=ob[:, cols], in_=dn[:TOK, cols])
                    if last:
                        nc.scalar.dma_start(out=y_d[rows, cols], in_=ob[:, cols])
                if not last:
                    nc.scalar.dma_start(out=y_d[rows, : H // 2], in_=ob[:, : H // 2])
                    nc.scalar.dma_start(out=y_d[rows, H // 2 :], in_=ob[:, H // 2 :])

    if not nc.is_finalized():
        nc.finalize()
    return nc


def _get_nc():
    if "nc" not in _cache:
        _cache["nc"] = _build_nc()
    return _cache["nc"]


def _pack_core(xs, q1, q3, q2):
    """Pack one core's slice into the kernel's DMA-ready layout.

    xs: pre-scaled x (float32, cast to bf16); q1/q3/q2: int8 weights.
    """
    xt = np.ascontiguousarray(
        xs.reshape(T_LOC, HC, P).transpose(2, 1, 0).reshape(P, HC * T_LOC)
    ).astype(BF16)
    # gate/up units: [e, u, p, (cs, {w1,w3}, f)]
    w1r = q1.reshape(E_LOC, HC, P, F)
    w3r = q3.reshape(E_LOC, HC, P, F)
    gu = np.stack([w1r, w3r], axis=3)               # [e, c, p, s, f]
    gu = gu.reshape(E_LOC, 4, 4, P, 2, F)           # [e, u, cs, p, s, f]
    gu = gu.transpose(0, 1, 3, 2, 4, 5).reshape(E_LOC, 4, P, UCOLS)
    # down units: [e, p, fc*H + h] split into 2 units of UCOLS
    dn = q2.reshape(E_LOC, FC, P, H).transpose(0, 2, 1, 3).reshape(E_LOC, P, 2, UCOLS)
    dn = dn.transpose(0, 2, 1, 3)                   # [e, 2, p, UCOLS]
    w = np.concatenate([gu, dn], axis=1)            # [e, 6, p, UCOLS] int8
    # merge unit pairs so each is one contiguous [128, 22528] int8 DMA:
    # (gu01 | gu23 | dn45)
    w = np.ascontiguousarray(
        w.reshape(E_LOC, 3, 2, P, UCOLS)
        .transpose(0, 1, 3, 2, 4)
        .reshape(E_LOC, 3, P, PAIR)
    )
    return xt, w


def _quant(w):
    s = float(np.abs(w).max()) / 127.0
    q = np.rint(w * (1.0 / s)).astype(np.int8)
    return q, s


def _make_in_maps(inputs):
    x = np.asarray(inputs["permuted_local_hidden_states"], dtype=np.float32)
    w1 = np.asarray(inputs["gate_proj"], dtype=np.float32)
    w3 = np.asarray(inputs["up_proj"], dtype=np.float32)
    w2 = np.asarray(inputs["down_proj"], dtype=np.float32)
    q1, s1 = _quant(w1)
    q3, s3 = _quant(w3)
    q2, s2 = _quant(w2)
    xs = x * np.float32(s1)
    in_maps = []
    for m in range(NCORES):
        xt, w = _pack_core(
            xs[m * T_LOC : (m + 1) * T_LOC],
            q1[m * E_LOC : (m + 1) * E_LOC],
            q3[m * E_LOC : (m + 1) * E_LOC],
            q2[m * E_LOC : (m + 1) * E_LOC],
        )
        in_maps.append({"xt": xt, "w": w})
    return in_maps, np.float32(s2 * s3 / s1)


def run(inputs, trace=False, **kwargs):
    """Run the SPMD kernel; returns (y_full, BassKernelResults)."""
    from concourse.bass_utils import run_bass_kernel_spmd

    nc = _get_nc()
    in_maps, post = _make_in_maps(inputs)
    res = run_bass_kernel_spmd(
        nc, in_maps, list(range(NCORES)), trace=trace, **kwargs
    )
    y = np.concatenate([res.results[m]["y"] for m in range(NCORES)], axis=0)
    return (y * post).astype(np.float32, copy=False), res


def kernel(**inputs):
    y, _ = run(inputs, trace=False)
    return y


# revision 4
# speedup vs baseline: 1.0515x; 1.0216x over previous
"""Grouped-GEMM MoE experts (E=64, H=2048, F=1408, 16 tokens/expert, SwiGLU),
expert-parallel across 8 Trainium2 NeuronCores - int8 weight streaming.

The bf16 baseline is DMA-bound (138.4 MB of weights/core at ~334-380 GB/s).
This version halves HBM traffic: weights are quantized to int8 on host
(global per-tensor symmetric scales; rel-err ~0.74% vs 2e-2 tolerance) and
dequantized on-chip to bf16 (int8->bf16 copy is exact for |v|<=127).

HW-measured rates that drive the design:
  - sync HWDGE ring: ~334 GB/s traced, 2.88 MB pair transfers (the sweet
    spot; per-unit 1.44 MB transfers measured 6% slower). Adding a SWDGE
    channel does NOT add bandwidth and cast-during-DMA costs 2x stream
    bytes -> both rejected after measurement.
  - DVE tensor_copy (CAST): 2 elem/cyc/lane = 245 G/s -> cols [0:6976)
  - ACT copy (ACTIVATE):    1 elem/cyc/lane = 153 G/s -> cols [6976:11264)
  - GPSIMD: ~4 cyc/elem and its SBUF-port contention slows DVE 3x -> unused
  - PE (FWL weight-stationary gate/up, inter-stationary down): ~178us busy
All four streams (DMA ~210us, ACT ~202, DVE ~192, PE ~178) are within ~15%,
and the wall time (252-262us traced across runs) sits ~45us above the DMA
span (head ramp + last-expert drain + barrier epilogue + in-order queue
coupling at silu/out-copies). Eleven variants were HW-measured against this
structure and ALL were slower or noise-ties:
  per-unit 1.44MB DMAs (257), quarter-piece head/tail (263), pipelined
  out-copies w/ w8=2 (303), w8=4/wt=4 (263), last-expert gu hoist (268),
  full software pipelining (357), VW=7104 strip rebalance (257), tail-pair
  split (259), dual HWDGE rings (308), SWDGE assist / cast-DMA (mb: no
  added bandwidth, 2x stream cost), two-pass gate-then-up (260).
This per-expert pair-streamed emission is the measured optimum; the Tile
scheduler handles it better than any manual reordering.

Scale bookkeeping (all folded outside the kernel):
  host:  x_tilde = x * s1 (bf16), weights stored as round(W/s) int8
  device: g = Q1^T x_tilde  (true gate values -> silu OK)
          u_raw = Q3^T x_tilde = (s1/s3) u
          y_raw = (silu(g) * u_raw)^T Q2
  host:  y = y_raw * (s2*s3/s1)
"""

import sys

if "/opt/trn_rl_repo" not in sys.path:
    sys.path.insert(0, "/opt/trn_rl_repo")

import numpy as np
import ml_dtypes

E, H, F = 64, 2048, 1408
TOK = 16                  # tokens per expert (uniform routing)
NCORES = 8
E_LOC = E // NCORES       # 8 experts per core
T_LOC = E_LOC * TOK       # 128 tokens per core
P = 128
HC = H // P               # 16 contraction chunks for gate/up
FC = F // P               # 11 contraction chunks for down
UCOLS = 2 * F * 4         # 11264 cols per weight unit
PAIR = 2 * UCOLS          # 22528 cols per int8 DMA pair (2.88 MB)
NFREE = 512               # matmul max free dim = one PSUM bank
BF16 = ml_dtypes.bfloat16
CW = 2 * F                # 2816 cols per gate/up h-chunk

VW = 6976                 # DVE dequant cols per unit (ACT gets the rest)

_cache = {}


def _build_nc():
    import concourse.mybir as mybir
    from concourse import bacc
    from concourse.tile import TileContext

    f32 = mybir.dt.float32
    bf16 = mybir.dt.bfloat16
    i8 = mybir.dt.int8
    AF = mybir.ActivationFunctionType

    nc = bacc.Bacc()
    xt_d = nc.declare_dram_parameter("xt", [P, HC * T_LOC], bf16, isOutput=False)
    w_d = nc.declare_dram_parameter("w", [E_LOC, 3, P, PAIR], i8, isOutput=False)
    y_d = nc.declare_dram_parameter("y", [T_LOC, H], f32, isOutput=True)

    with TileContext(nc) as tc:
        with (
            tc.tile_pool(name="xs", bufs=1) as xs,
            tc.tile_pool(name="w8", bufs=3) as w8,
            tc.tile_pool(name="wt", bufs=5) as wt,
            tc.tile_pool(name="acts", bufs=2) as acts,
            tc.tile_pool(name="ps_gu", bufs=2, space="PSUM") as ps_gu,
            tc.tile_pool(name="ps_dn", bufs=1, space="PSUM") as ps_dn,
        ):
            # xt on the scalar HWDGE ring so the weight stream (sync ring)
            # starts immediately; xt is only needed a few us in.
            xt = xs.tile([P, HC * T_LOC], bf16)
            nc.scalar.dma_start(out=xt[:], in_=xt_d[:, :])

            def convert(dst, src, off, lo, hi):
                """Dequant dst[:, lo:hi] from src[:, off+lo : off+hi]:
                DVE below VW, ACT above (measured 2 vs 1 elem/cyc/lane)."""
                if lo < VW:
                    m = min(hi, VW)
                    nc.vector.tensor_copy(out=dst[:, lo:m], in_=src[:, off + lo : off + m])
                if hi > VW:
                    m = max(lo, VW)
                    nc.scalar.copy(out=dst[:, m:hi], in_=src[:, off + m : off + hi])

            for e in range(E_LOC):
                last = e == E_LOC - 1
                # int8 weight stream: 3 pair transfers of [128, 22528] per
                # expert (gu01 | gu23 | dn45), each dequantized into 2 bf16
                # units. The head pair of expert 0 and the tail pair of the
                # last expert stream in quarter pieces so the pipeline ramps
                # in ~2us and the final matmuls chase the last bytes.
                units = []
                for j in range(3):
                    p8 = w8.tile([P, PAIR], i8, tag="p8")
                    fine = False
                    if fine:
                        q = PAIR // 4  # 5632
                        for k in range(4):
                            nc.sync.dma_start(
                                out=p8[:, k * q : (k + 1) * q],
                                in_=w_d[e, j, :, k * q : (k + 1) * q],
                            )
                            if k % 2 == 0:
                                t = wt.tile([P, UCOLS], bf16, tag="w")
                            # strips aligned to each 5632-col piece:
                            # DVE [0:3584]+[5632:9216], ACT [3584:5632]+[9216:]
                            if k % 2 == 0:
                                nc.vector.tensor_copy(
                                    out=t[:, :3584], in_=p8[:, k * q : k * q + 3584]
                                )
                                nc.scalar.copy(
                                    out=t[:, 3584:q], in_=p8[:, k * q + 3584 : (k + 1) * q]
                                )
                            else:
                                nc.vector.tensor_copy(
                                    out=t[:, q:9216], in_=p8[:, k * q : k * q + 3584]
                                )
                                nc.scalar.copy(
                                    out=t[:, 9216:], in_=p8[:, k * q + 3584 : (k + 1) * q]
                                )
                                units.append(t)
                    else:
                        nc.sync.dma_start(out=p8[:], in_=w_d[e, j, :, :])
                        for half in range(2):
                            t = wt.tile([P, UCOLS], bf16, tag="w")
                            convert(t, p8, half * UCOLS, 0, UCOLS)
                            units.append(t)
                gu_map = [(units[c // 4], (c % 4) * CW) for c in range(HC)]
                dn_units = ((units[4], 0, UCOLS), (units[5], UCOLS, PAIR))

                # gate/up: all FC output chunks share one PSUM bank per
                # tensor; only the first matmul into the bank clears it
                # (start=True), later chunks overwrite via has_written.
                gt = ps_gu.tile([P, FC * TOK], f32, tag="gt")
                ut = ps_gu.tile([P, FC * TOK], f32, tag="ut")
                rhs_e = e * TOK
                for c in range(HC):
                    wu, base = gu_map[c]
                    rhs = xt[:, c * T_LOC + rhs_e : c * T_LOC + rhs_e + TOK]
                    first = c == 0
                    final = c == HC - 1
                    w1o = base
                    w3o = base + F
                    for fc in range(FC):
                        nc.tensor.matmul(
                            gt[:, fc * TOK : (fc + 1) * TOK],
                            wu[:, w1o + fc * P : w1o + (fc + 1) * P],
                            rhs,
                            start=(first and fc == 0),
                            stop=(final and fc == FC - 1),
                            skip_group_check=True,
                        )
                    for fc in range(FC):
                        nc.tensor.matmul(
                            ut[:, fc * TOK : (fc + 1) * TOK],
                            wu[:, w3o + fc * P : w3o + (fc + 1) * P],
                            rhs,
                            start=(first and fc == 0),
                            stop=(final and fc == FC - 1),
                            skip_group_check=True,
                        )

                gs = acts.tile([P, FC * TOK], f32, tag="gs")
                it = acts.tile([P, FC * TOK], bf16, tag="it")
                nc.scalar.activation(gs[:], gt[:], AF.Silu)
                nc.vector.tensor_mul(it[:], gs[:], ut[:])

                def dn_src(col):
                    for t, lo, hi in dn_units:
                        if lo <= col < hi:
                            return t, col - lo
                    raise AssertionError

                dn = ps_dn.tile([P, H], f32, tag="dn")
                for fc in range(FC):
                    for nt in range(H // NFREE):
                        col = fc * H + nt * NFREE
                        wu, off = dn_src(col)
                        nc.tensor.matmul(
                            dn[:TOK, nt * NFREE : (nt + 1) * NFREE],
                            it[:, fc * TOK : (fc + 1) * TOK],
                            wu[:, off : off + NFREE],
                            start=(fc == 0),
                            stop=(fc == FC - 1),
                        )

                # copy out per 512-col slice (PSUM bank granularity) so the
                # copies overlap the last f-chunk's matmuls, alternating
                # DVE/ACT to split the load; y DMAs go on the scalar HWDGE
                # ring so they never stall the sync weight ring.
                ob = acts.tile([TOK, H], f32, tag="ob")
                rows = slice(e * TOK, (e + 1) * TOK)
                for nt in range(H // NFREE):
                    cols = slice(nt * NFREE, (nt + 1) * NFREE)
                    if nt % 2 == 0:
                        nc.vector.tensor_copy(out=ob[:, cols], in_=dn[:TOK, cols])
                    else:
                        nc.scalar.copy(out=ob[:, cols], in_=dn[:TOK, cols])
                    if last:
                        nc.scalar.dma_start(out=y_d[rows, cols], in_=ob[:, cols])
                if not last:
                    nc.scalar.dma_start(out=y_d[rows, : H // 2], in_=ob[:, : H // 2])
                    nc.scalar.dma_start(out=y_d[rows, H // 2 :], in_=ob[:, H // 2 :])

    if not nc.is_finalized():
        nc.finalize()
    return nc


def _get_nc():
    if "nc" not in _cache:
        _cache["nc"] = _build_nc()
    return _cache["nc"]


def _pack_core(xs, q1, q3, q2):
    """Pack one core's slice into the kernel's DMA-ready layout.

    xs: pre-scaled x (float32, cast to bf16); q1/q3/q2: int8 weights.
    """
    xt = np.ascontiguousarray(
        xs.reshape(T_LOC, HC, P).transpose(2, 1, 0).reshape(P, HC * T_LOC)
    ).astype(BF16)
    # gate/up units: [e, u, p, (cs, {w1,w3}, f)]
    w1r = q1.reshape(E_LOC, HC, P, F)
    w3r = q3.reshape(E_LOC, HC, P, F)
    gu = np.stack([w1r, w3r], axis=3)               # [e, c, p, s, f]
    gu = gu.reshape(E_LOC, 4, 4, P, 2, F)           # [e, u, cs, p, s, f]
    gu = gu.transpose(0, 1, 3, 2, 4, 5).reshape(E_LOC, 4, P, UCOLS)
    # down units: [e, p, fc*H + h] split into 2 units of UCOLS
    dn = q2.reshape(E_LOC, FC, P, H).transpose(0, 2, 1, 3).reshape(E_LOC, P, 2, UCOLS)
    dn = dn.transpose(0, 2, 1, 3)                   # [e, 2, p, UCOLS]
    w = np.concatenate([gu, dn], axis=1)            # [e, 6, p, UCOLS] int8
    # merge unit pairs so each is one contiguous [128, 22528] int8 DMA:
    # (gu01 | gu23 | dn45)
    w = np.ascontiguousarray(
        w.reshape(E_LOC, 3, 2, P, UCOLS)
        .transpose(0, 1, 3, 2, 4)
        .reshape(E_LOC, 3, P, PAIR)
    )
    return xt, w


def _quant(w):
    s = float(np.abs(w).max()) / 127.0
    q = np.rint(w * (1.0 / s)).astype(np.int8)
    return q, s


def _make_in_maps(inputs):
    x = np.asarray(inputs["permuted_local_hidden_states"], dtype=np.float32)
    w1 = np.asarray(inputs["gate_proj"], dtype=np.float32)
    w3 = np.asarray(inputs["up_proj"], dtype=np.float32)
    w2 = np.asarray(inputs["down_proj"], dtype=np.float32)
    q1, s1 = _quant(w1)
    q3, s3 = _quant(w3)
    q2, s2 = _quant(w2)
    xs = x * np.float32(s1)
    in_maps = []
    for m in range(NCORES):
        xt, w = _pack_core(
            xs[m * T_LOC : (m + 1) * T_LOC],
            q1[m * E_LOC : (m + 1) * E_LOC],
            q3[m * E_LOC : (m + 1) * E_LOC],
            q2[m * E_LOC : (m + 1) * E_LOC],
        )
        in_maps.append({"xt": xt, "w": w})
    return in_maps, np.float32(s2 * s3 / s1)


def run(inputs, trace=False, **kwargs):
    """Run the SPMD kernel; returns (y_full, BassKernelResults)."""
    from concourse.bass_utils import run_bass_kernel_spmd

    nc = _get_nc()
    in_maps, post = _make_in_maps(inputs)
    res = run_bass_kernel_spmd(
        nc, in_maps, list(range(NCORES)), trace=trace, **kwargs
    )
    y = np.concatenate([res.results[m]["y"] for m in range(NCORES)], axis=0)
    return (y * post).astype(np.float32, copy=False), res


def kernel(**inputs):
    y, _ = run(inputs, trace=False)
    return y
